# revision 1
# baseline (speedup 1.0000x reference)
"""GRU decoder with dot attention (nn_Decoder) on 8 Trainium2 cores.

Strategy: data-parallel over batch (8 samples/core). Per core:
  Phase 1 (recurrence): GRU scan in transposed layout (H on partitions).
    gh^T = W_hh^T-tiles (stationary) @ h^T, gates on (128, 4x8) tiles.
    Input-side gates gi = embed@W_ih.T + biases are a 32-row table gathered
    on host and streamed from DRAM per step.
  Phase 2 (attention): per sample, scores = H_allT.T @ encT (fp32r matmuls),
    additive src-len mask via K=1 matmul, softmax along free dim (DVE max,
    ACT exp with fused row-sum, normalize), PE-transpose of the weights,
    ctx^T = encN.T @ w^T, then one fused FC with bias folded into the
    PSUM->SBUF copy. Output unshard + trg_len padding on host.
"""

import sys

for _p in ("/opt/trn_rl_repo", "/root/.axon_site/_ro/trn_rl_repo"):
    if _p not in sys.path:
        sys.path.append(_p)

import numpy as np
from contextlib import ExitStack

import concourse.bass as bass
import concourse.tile as tile
from concourse import bacc, mybir
from concourse import bass_utils
from concourse.masks import make_identity

F32 = mybir.dt.float32
F32R = mybir.dt.float32r
AF = mybir.ActivationFunctionType
AX = mybir.AxisListType

B, TT, ST, H, E, V, O = 64, 256, 1024, 512, 512, 32, 31
NCORES = 8
BS = B // NCORES  # 8 samples per core
H3 = 3 * H        # 1536
NEG = -1.0e9

_cache = {}


def _build(tt=TT, dbg=False):
    nc = bacc.Bacc("TRN2", target_bir_lowering=False, debug=False)

    wt_d = nc.dram_tensor("wt", [4, 128, H3], F32, kind="ExternalInput")
    gid_d = nc.dram_tensor("gid", [tt, 128, 96], F32, kind="ExternalInput")
    h0_d = nc.dram_tensor("h0", [128, 4, BS], F32, kind="ExternalInput")
    bhn_d = nc.dram_tensor("bhn", [128, 4, BS], F32, kind="ExternalInput")
    mb_d = nc.dram_tensor("maskb", [1, BS * ST], F32, kind="ExternalInput")
    encT_d = nc.dram_tensor("encT", [BS, 4, 128, ST], F32, kind="ExternalInput")
    encN_d = nc.dram_tensor("encN", [BS, 8, 128, H], F32, kind="ExternalInput")
    fcw_d = nc.dram_tensor("fcw", [8, 128, O], F32, kind="ExternalInput")
    fcb_d = nc.dram_tensor("fcb", [O, 1], F32, kind="ExternalInput")
    outT_d = nc.dram_tensor("outT", [O, BS * tt], F32, kind="ExternalOutput")
    if dbg:
        zh_d = nc.dram_tensor("zh", [128, 4, BS, tt], F32, kind="ExternalOutput")
        zc_d = nc.dram_tensor("zc", [128, 4, BS, tt], F32, kind="ExternalOutput")

    ntt = tt // 128  # t-tiles for attention (2)

    with tile.TileContext(nc) as tc, ExitStack() as ctx:
        singles = ctx.enter_context(tc.tile_pool(name="singles", bufs=1))

        wt_sb = singles.tile([128, 4, H3], F32)
        nc.sync.dma_start(out=wt_sb, in_=wt_d.ap().rearrange("c p m -> p c m"))
        h0_sb = singles.tile([128, 4, BS], F32)
        nc.sync.dma_start(out=h0_sb, in_=h0_d.ap())
        # b_hh n-gate bias, replicated across the batch dim: [p, c, b]
        bhn_sb = singles.tile([128, 4, BS], F32)
        nc.sync.dma_start(out=bhn_sb, in_=bhn_d.ap())
        mb_sb = singles.tile([1, BS * ST], F32)
        nc.sync.dma_start(out=mb_sb, in_=mb_d.ap())
        fcw_sb = singles.tile([128, 8, O], F32)
        nc.sync.dma_start(out=fcw_sb, in_=fcw_d.ap().rearrange("c p o -> p c o"))
        fcb_sb = singles.tile([O, 1], F32)
        nc.sync.dma_start(out=fcb_sb, in_=fcb_d.ap())
        ident = singles.tile([128, 128], F32)
        make_identity(nc, ident)
        ones1 = singles.tile([1, 128], F32)
        nc.vector.memset(ones1, 1.0)

        # H_all^T and ctx^T, layout [p, chunk, b, t]
        Zh = singles.tile([128, 4, BS, tt], F32)
        Zc = singles.tile([128, 4, BS, tt], F32)

        # ---------------- Phase 1: GRU recurrence ----------------
        with tc.tile_pool(name="ghp", bufs=2, space="PSUM") as ghp, \
             tc.tile_pool(name="gip", bufs=4) as gip, \
             tc.tile_pool(name="gates", bufs=3) as gp:
            for t in range(tt):
                gi_t = gip.tile([128, 12, BS], F32, tag="gi")
                nc.sync.dma_start(
                    out=gi_t,
                    in_=gid_d.ap()[t].rearrange("p (j b) -> p j b", j=12),
                )
                gh = ghp.tile([128, 12, BS], F32, tag="gh")
                hprev = h0_sb[:, :, :] if t == 0 else Zh[:, :, :, t - 1]
                for j in range(12):
                    for c in range(4):
                        nc.tensor.matmul(
                            gh[:, j, :],
                            lhsT=wt_sb[:, c, 128 * j:128 * (j + 1)],
                            rhs=hprev[:, c, :],
                            start=(c == 0),
                            stop=(c == 3),
                        )
                # r|z = sigmoid(gh_rz + gi_rz)
                srz = gp.tile([128, 8, BS], F32, tag="srz")
                nc.vector.tensor_add(srz, gh[:, 0:8, :], gi_t[:, 0:8, :])
                rz = gp.tile([128, 8, BS], F32, tag="rz")
                nc.scalar.activation(rz, srz, AF.Sigmoid)
                # n = tanh(gi_n + r * (gh_n + b_hn))
                gn = gp.tile([128, 4, BS], F32, tag="gn")
                nc.vector.tensor_add(gn, gh[:, 8:12, :], bhn_sb)
                mm_ = gp.tile([128, 4, BS], F32, tag="mm")
                nc.vector.tensor_mul(mm_, rz[:, 0:4, :], gn)
                an = gp.tile([128, 4, BS], F32, tag="an")
                nc.vector.tensor_add(an, mm_, gi_t[:, 8:12, :])
                nn = gp.tile([128, 4, BS], F32, tag="nn")
                nc.scalar.activation(nn, an, AF.Tanh)
                # h' = n + z * (h - n)
                ee = gp.tile([128, 4, BS], F32, tag="ee")
                nc.vector.tensor_sub(ee, hprev, nn)
                ff = gp.tile([128, 4, BS], F32, tag="ff")
                nc.vector.tensor_mul(ff, rz[:, 4:8, :], ee)
                nc.vector.tensor_add(Zh[:, :, :, t], nn, ff)

        # ---------------- Phase 2: attention ----------------
        with tc.tile_pool(name="scp", bufs=1, space="PSUM") as scp, \
             tc.tile_pool(name="tpp", bufs=2, space="PSUM") as tpp, \
             tc.tile_pool(name="cxp", bufs=1, space="PSUM") as cxp, \
             tc.tile_pool(name="ep", bufs=10) as ep, \
             tc.tile_pool(name="ap_", bufs=2) as ap_:
            for b in range(BS):
                # scores (t-part, s-free), masked via K=1 matmul
                Sp = scp.tile([128, ntt, ST], F32, tag="sp")
                encT_t = {}
                for c in range(4):
                    for ns in range(2):
                        et = ep.tile([128, 512], F32, tag="et")
                        nc.sync.dma_start(
                            out=et, in_=encT_d.ap()[b, c, :, 512 * ns:512 * (ns + 1)]
                        )
                        encT_t[c, ns] = et
                for m in range(ntt):
                    for ns in range(2):
                        dst = Sp[:, m, 512 * ns:512 * (ns + 1)]
                        for c in range(4):
                            nc.tensor.matmul(
                                dst,
                                lhsT=Zh[:, c, b, 128 * m:128 * (m + 1)],
                                rhs=encT_t[c, ns],
                                start=(c == 0),
                                stop=False,
                            )
                        nc.tensor.matmul(
                            dst,
                            lhsT=ones1,
                            rhs=mb_sb[0:1, b * ST + 512 * ns:b * ST + 512 * (ns + 1)],
                            start=False,
                            stop=True,
                        )
                # softmax along free dim
                mx = ap_.tile([128, ntt], F32, tag="mx")
                for m in range(ntt):
                    nc.vector.tensor_reduce(
                        mx[:, m:m + 1], Sp[:, m, :], axis=AX.X, op=mybir.AluOpType.max
                    )
                nmx = ap_.tile([128, ntt], F32, tag="nmx")
                nc.vector.tensor_scalar_mul(nmx, mx, -1.0)
                Eb = ap_.tile([128, ntt, ST], F32, tag="eb")
                sume = ap_.tile([128, ntt], F32, tag="sume")
                for m in range(ntt):
                    nc.scalar.activation(
                        Eb[:, m, :], Sp[:, m, :], AF.Exp,
                        bias=nmx[:, m:m + 1], scale=1.0,
                        accum_out=sume[:, m:m + 1],
                    )
                rec = ap_.tile([128, ntt], F32, tag="rec")
                nc.vector.reciprocal(rec, sume)
                for m in range(ntt):
                    nc.vector.tensor_scalar_mul(
                        Eb[:, m, :], Eb[:, m, :], rec[:, m:m + 1]
                    )
                # transpose weights: (t-part, s-free) -> (s-part, t-free)
                WT = ap_.tile([128, 8, ntt * 128], F32, tag="wt")
                for cs in range(8):
                    for m in range(ntt):
                        tp_ = tpp.tile([128, 128], F32, tag="tp")
                        nc.tensor.transpose(
                            tp_, Eb[:, m, 128 * cs:128 * (cs + 1)], ident
                        )
                        nc.vector.tensor_copy(
                            WT[:, cs, 128 * m:128 * (m + 1)], tp_
                        )
                # ctx^T = encN.T @ WT
                Cp = cxp.tile([128, 4, tt], F32, tag="cp")
                encN_t = {}
                for cs in range(8):
                    en = ep.tile([128, H], F32, tag="en")
                    nc.sync.dma_start(out=en, in_=encN_d.ap()[b, cs])
                    encN_t[cs] = en
                for m2 in range(4):
                    for cs in range(8):
                        nc.tensor.matmul(
                            Cp[:, m2, :],
                            lhsT=encN_t[cs][:, 128 * m2:128 * (m2 + 1)],
                            rhs=WT[:, cs, :],
                            start=(cs == 0),
                            stop=(cs == 7),
                        )
                for m2 in range(4):
                    nc.vector.tensor_copy(Zc[:, m2, b, :], Cp[:, m2, :])

        # ---------------- Phase 3: FC ----------------
        with tc.tile_pool(name="fcp", bufs=1, space="PSUM") as fcp_pool, \
             tc.tile_pool(name="fop", bufs=2) as fop:
            Fp = fcp_pool.tile([O, BS * tt], F32)
            for nb in range(BS * tt // 512):
                for cc in range(8):
                    zsrc = Zh if cc < 4 else Zc
                    rhs = zsrc[:, cc % 4, :, :].rearrange("p b t -> p (b t)")
                    nc.tensor.matmul(
                        Fp[:, 512 * nb:512 * (nb + 1)],
                        lhsT=fcw_sb[:, cc, :],
                        rhs=rhs[:, 512 * nb:512 * (nb + 1)],
                        start=(cc == 0),
                        stop=(cc == 7),
                    )
            outsb = fop.tile([O, BS * tt], F32)
            nc.scalar.activation(outsb, Fp, AF.Identity, bias=fcb_sb[:, 0:1], scale=1.0)
            nc.sync.dma_start(out=outT_d.ap(), in_=outsb)
            if dbg:
                nc.sync.dma_start(out=zh_d.ap(), in_=Zh)
                nc.sync.dma_start(out=zc_d.ap(), in_=Zc)

    nc.compile()
    return nc


def _prep_core(k, trg, trg_len, source_len, enc, h0v, G, wt, fcw, fcb, tt):
    s = slice(BS * k, BS * (k + 1))
    gi_core = G[trg[s, :tt]]  # (BS, tt, 1536)
    gid = (
        gi_core.transpose(1, 2, 0)
        .reshape(tt, 12, 128, BS)
        .transpose(0, 2, 1, 3)
        .reshape(tt, 128, 96)
    )
    h0c = np.ascontiguousarray(
        h0v[s].T.reshape(4, 128, BS).transpose(1, 0, 2)
    )  # (128, 4, BS)
    maskb = np.where(
        np.arange(ST)[None, :] < source_len[s, None], 0.0, NEG
    ).astype(np.float32).reshape(1, BS * ST)
    encs = enc[s]  # (BS, ST, H)
    encT = np.ascontiguousarray(
        encs.transpose(0, 2, 1).reshape(BS, 4, 128, ST)
    )
    encN = np.ascontiguousarray(encs.reshape(BS, 8, 128, H))
    return {
        "wt": wt,
        "gid": np.ascontiguousarray(gid),
        "h0": h0c,
        "maskb": maskb,
        "encT": encT,
        "encN": encN,
        "fcw": fcw,
        "fcb": fcb,
    }


def host_prep(inp, tt=TT):
    trg = np.asarray(inp["trg_inputs"]).astype(np.int64)
    trg_len = np.asarray(inp["trg_len"]).astype(np.int64)
    source_len = np.asarray(inp["source_len"]).astype(np.int64)
    enc = np.asarray(inp["encoder_outputs"], dtype=np.float32)
    h0v = np.asarray(inp["encoder_last_hidden"], dtype=np.float32)[0]
    embed = np.asarray(inp["embed"], dtype=np.float32)
    W_ih = np.asarray(inp["W_ih"], dtype=np.float32)
    W_hh = np.asarray(inp["W_hh"], dtype=np.float32)
    b_ih = np.asarray(inp["b_ih"], dtype=np.float32)
    b_hh = np.asarray(inp["b_hh"], dtype=np.float32)
    fc_W = np.asarray(inp["fc_W"], dtype=np.float32)
    fc_b = np.asarray(inp["fc_b"], dtype=np.float32)

    # fold b_ih fully into the token gate table; b_hh only for the r/z
    # blocks (the n-block's b_hn sits inside the r-product in the GRU cell)
    bh_rz = b_hh.copy()
    bh_rz[2 * H:] = 0.0
    G = (embed @ W_ih.T + b_ih + bh_rz).astype(np.float32)  # (V, 3H)
    bhn = np.ascontiguousarray(
        np.broadcast_to(b_hh[2 * H:].reshape(4, 128).T[:, :, None], (128, 4, BS))
    )  # (128, 4, BS)
    wt = np.ascontiguousarray(W_hh.T.reshape(4, 128, H3))
    fcw = np.ascontiguousarray(fc_W.T.reshape(8, 128, O))
    fcb = np.ascontiguousarray(fc_b.reshape(O, 1))

    in_maps = []
    for k in range(NCORES):
        m = _prep_core(k, trg, trg_len, source_len, enc, h0v, G, wt, fcw, fcb, tt)
        m["bhn"] = bhn
        in_maps.append(m)
    return in_maps


def kernel(trg_inputs, trg_len, source_len, encoder_outputs,
           encoder_last_hidden, embed, W_ih, W_hh, b_ih, b_hh, fc_W, fc_b,
           tt=TT):
    inp = dict(
        trg_inputs=trg_inputs, trg_len=trg_len, source_len=source_len,
        encoder_outputs=encoder_outputs, encoder_last_hidden=encoder_last_hidden,
        embed=embed, W_ih=W_ih, W_hh=W_hh, b_ih=b_ih, b_hh=b_hh,
        fc_W=fc_W, fc_b=fc_b,
    )
    trg_len = np.asarray(trg_len)
    in_maps = host_prep(inp, tt)

    if tt not in _cache:
        _cache[tt] = _build(tt)
    nc = _cache[tt]

    res = bass_utils.run_bass_kernel_spmd(nc, in_maps, core_ids=list(range(NCORES)))

    out = np.empty((B, tt, O), np.float32)
    for k in range(NCORES):
        o = res.results[k]["outT"]  # (O, BS*tt)
        out[BS * k:BS * (k + 1)] = o.reshape(O, BS, tt).transpose(1, 2, 0)
    tmask = np.arange(tt)[None, :] < trg_len[:, None]
    out = np.where(tmask[:, :, None], out, 0.0).astype(np.float32)
    return out

